# revision 17
# baseline (speedup 1.0000x reference)
"""Trainium2 Bass kernel for MoRAttention (sparse selective-KV GQA attention).

Math: the reference's argsort/gather of active keys == dense attention with
mask = active[k] & (pos[k] <= pos[q]) (softmax is permutation invariant along
keys; q columns are independent). We permute the q axis on the host to
actives-first order, so the first SA columns of the permuted hidden state ARE
the compacted keys: K/V projections read a prefix of the same SBUF tensor the
Q projection reads — no separate gathered-x input. The host inverse-permutes
the output columns.

Causal structure over sorted keys/queries lets us statically skip dead
(k-chunk, q-span) tiles; tiles that are computed but not fully valid get a
multiplicative mask precomputed on the host.

Sharding: 8 cores = 2 batches x 4 kv-groups. Core (b, g) computes q-heads
[4g, 4g+4) + kv-head g of batch b, producing a partial o_proj output
out^T [D, S]; the host sums the 4 partials per batch.

All matmul operands bf16 (same PE rate as f32r, half the DMA/SBUF traffic),
f32 PSUM. Pipeline: K, V, Q0, B0|Q1, B1|Q2, B2|Q3, B3, C with attnV/colsum
staggered one k-chunk behind scores so PE never waits on exp, and input DMAs
striped across the three DGE rings (SP, Act, Pool) in consumption order.
"""

import numpy as np

S, D, HD = 1024, 2048, 128
NH = 4           # q heads per core
DC = D // 128    # contraction chunks
SCALE = HD ** -0.5

TRACE = False
LAST_EXEC_NS = None
LAST_RESULTS = None

_NC_CACHE = {}


def _build_nc(meta):
    import concourse.mybir as mybir
    from concourse import bacc
    from concourse.tile import TileContext
    from contextlib import ExitStack

    SAC, qa_kc, span01_kcs, mask_runs, nstr = meta
    SA = SAC * 128
    NSTR = max(1, nstr)
    runs_by_kc = {}
    for (kc, qt0, n, idx0) in mask_runs:
        runs_by_kc.setdefault(kc, []).append((qt0, n, idx0))

    f32 = mybir.dt.float32
    bf16 = mybir.dt.bfloat16
    f8 = mybir.dt.float8e4
    DR = mybir.MatmulPerfMode.DoubleRow
    Exp = mybir.ActivationFunctionType.Exp

    nc = bacc.Bacc("TRN2", target_bir_lowering=False, debug=False)

    xh_d = nc.dram_tensor("xh", [128, DC * S], f8, kind="ExternalInput")
    xl_d = nc.dram_tensor("xl", [128, DC * S], f8, kind="ExternalInput")
    wqh_d = nc.dram_tensor("wqh", [128, DC * 512], f8, kind="ExternalInput")
    wql_d = nc.dram_tensor("wql", [128, DC * 512], f8, kind="ExternalInput")
    wkh_d = nc.dram_tensor("wkh", [128, DC * 128], f8, kind="ExternalInput")
    wkl_d = nc.dram_tensor("wkl", [128, DC * 128], f8, kind="ExternalInput")
    wvh_d = nc.dram_tensor("wvh", [128, DC * 128], f8, kind="ExternalInput")
    wvl_d = nc.dram_tensor("wvl", [128, DC * 128], f8, kind="ExternalInput")
    wo_d = nc.dram_tensor("wo", [128, NH * D], bf16, kind="ExternalInput")
    cq_d = nc.dram_tensor("cq", [128, S], bf16, kind="ExternalInput")
    sq_d = nc.dram_tensor("sq", [128, S], bf16, kind="ExternalInput")
    mk_d = nc.dram_tensor("mk", [128, NSTR * 128], bf16, kind="ExternalInput")
    out_d = nc.dram_tensor("out", [128, DC * S], bf16, kind="ExternalOutput")

    with TileContext(nc) as tc, ExitStack() as ctx:
        singles = ctx.enter_context(tc.tile_pool(name="singles", bufs=1))
        persist = ctx.enter_context(tc.tile_pool(name="persist", bufs=1))

        ones_tmp = singles.tile([128, 128], f32)
        nc.vector.memset(ones_tmp, 1.0)
        ones128 = singles.tile([128, 128], bf16)
        nc.vector.tensor_copy(ones128, ones_tmp)

        xh_sb = persist.tile([128, DC * S], f8, tag="xh")
        xl_sb = persist.tile([128, DC * S], f8, tag="xl")
        wqh_sb = persist.tile([128, DC * 512], f8, tag="wqh")
        wql_sb = persist.tile([128, DC * 512], f8, tag="wql")
        wkh_sb = persist.tile([128, DC * 128], f8, tag="wkh")
        wkl_sb = persist.tile([128, DC * 128], f8, tag="wkl")
        wvh_sb = persist.tile([128, DC * 128], f8, tag="wvh")
        wvl_sb = persist.tile([128, DC * 128], f8, tag="wvl")
        # paired-chunk views for DoubleRow matmuls (dim1 = k-tile pair)
        xh3 = xh_sb.rearrange("p (c s) -> p c s", c=DC)
        xl3 = xl_sb.rearrange("p (c s) -> p c s", c=DC)
        wqh3 = wqh_sb.rearrange("p (c f) -> p c f", c=DC)
        wql3 = wql_sb.rearrange("p (c f) -> p c f", c=DC)
        wkh3 = wkh_sb.rearrange("p (c f) -> p c f", c=DC)
        wkl3 = wkl_sb.rearrange("p (c f) -> p c f", c=DC)
        wvh3 = wvh_sb.rearrange("p (c f) -> p c f", c=DC)
        wvl3 = wvl_sb.rearrange("p (c f) -> p c f", c=DC)
        wo_sb = persist.tile([128, NH * D], bf16, tag="wo")
        cq_sb = persist.tile([128, S], bf16, tag="cq")
        sq_sb = persist.tile([128, S], bf16, tag="sq")
        mk_sb = persist.tile([128, NSTR * 128], bf16, tag="mk")

        kT = persist.tile([128, SA], bf16, tag="kT")
        vn = persist.tile([128, SA], bf16, tag="vn")
        qT = [persist.tile([128, S], bf16, tag=f"qT{h}", name=f"qT{h}") for h in range(NH)]
        attn = [persist.tile([128, S], bf16, tag=f"attn{h}", name=f"attn{h}") for h in range(NH)]

        # ---- input DMAs striped across the 3 DGE rings by consumption order.
        # K/V chains consume xs[:, dc*S : dc*S+SA] (the compacted-key prefix of
        # each chunk) first; Q chains then read the full chunks; wq is needed
        # from the Q0 chain on.
        # hwdge rings (sync/scalar) carry everything consumed early; the
        # software-DGE gpsimd ring only carries late-consumed bulk (q-only
        # chunk tails, o-proj weights).
        hw = [nc.sync, nc.scalar]
        # K/V weights first (hi then lo), then x prefixes in pass-consumption
        # order (hi before lo), then rope tables/masks, chunk tails, q weights
        nc.scalar.dma_start(out=wkh_sb, in_=wkh_d[:, :])
        nc.sync.dma_start(out=wvh_sb, in_=wvh_d[:, :])
        for dc in range(DC):
            hw[dc % 2].dma_start(
                out=xh_sb[:, dc * S: dc * S + SA], in_=xh_d[:, dc * S: dc * S + SA]
            )
        nc.scalar.dma_start(out=wkl_sb, in_=wkl_d[:, :])
        nc.sync.dma_start(out=wvl_sb, in_=wvl_d[:, :])
        for dc in range(DC):
            hw[dc % 2].dma_start(
                out=xl_sb[:, dc * S: dc * S + SA], in_=xl_d[:, dc * S: dc * S + SA]
            )
        nc.sync.dma_start(out=cq_sb, in_=cq_d[:, :])
        nc.scalar.dma_start(out=sq_sb, in_=sq_d[:, :])
        nc.scalar.dma_start(out=mk_sb, in_=mk_d[:, :])
        for dc in range(DC):  # q-only tails of each chunk
            hw[dc % 2].dma_start(
                out=xh_sb[:, dc * S + SA: (dc + 1) * S],
                in_=xh_d[:, dc * S + SA: (dc + 1) * S],
            )
        for i in range(4):  # q-proj weights (hi first)
            w = DC * 512 // 4
            hw[i % 2].dma_start(
                out=wqh_sb[:, i * w:(i + 1) * w], in_=wqh_d[:, i * w:(i + 1) * w]
            )
        for dc in range(DC):  # lo tails on the software-DGE ring (late use)
            nc.gpsimd.dma_start(
                out=xl_sb[:, dc * S + SA: (dc + 1) * S],
                in_=xl_d[:, dc * S + SA: (dc + 1) * S],
            )
        for i in range(4):
            w = DC * 512 // 4
            nc.gpsimd.dma_start(
                out=wql_sb[:, i * w:(i + 1) * w], in_=wql_d[:, i * w:(i + 1) * w]
            )
        for i in range(2):  # o-proj weights, needed only in phase C
            w = NH * D // 2
            nc.gpsimd.dma_start(
                out=wo_sb[:, i * w:(i + 1) * w], in_=wo_d[:, i * w:(i + 1) * w]
            )

        def rope(psum, cos_t, sin_t, dst, w, pool):
            # dst = psum*cos + rot_half(psum)*sin2  (sin2 pre-arranged so a
            # plain half-swap after the multiply gives rot_half()*sin)
            pc = pool.tile([128, w], bf16, tag="ropec")
            ps_ = pool.tile([128, w], bf16, tag="ropes")
            pw = pool.tile([128, w], bf16, tag="ropew")
            nc.vector.tensor_mul(pc, psum, cos_t)
            nc.vector.tensor_mul(ps_, psum, sin_t)
            nc.sync.dma_start(out=pw[0:64, :], in_=ps_[64:128, :])
            nc.sync.dma_start(out=pw[64:128, :], in_=ps_[0:64, :])
            nc.vector.tensor_add(dst, pc, pw)

        def dr_chain(psum_ap, wh3, wl3, f0, f1, c0, c1):
            # psum_ap += (wh+wl)^T (xh+xl), dropping the lo*lo term; each
            # DoubleRow matmul contracts a pair of 128-chunks at 2x rate
            passes = ((wh3, xh3), (wh3, xl3), (wl3, xh3))
            n = 0
            last = 3 * (DC // 2) - 1
            for (w3, x3) in passes:
                for pr in range(DC // 2):
                    nc.tensor.matmul(
                        psum_ap,
                        lhsT=w3[:, 2 * pr:2 * pr + 2, f0:f1],
                        rhs=x3[:, 2 * pr:2 * pr + 2, c0:c1],
                        start=(n == 0), stop=(n == last),
                        perf_mode=DR,
                    )
                    n += 1

        # ================= Phase A: K, V =================
        with tc.tile_pool(name="pkv", bufs=2, space="PSUM") as pkv, \
             tc.tile_pool(name="ropep", bufs=2) as ropep:
            psum_k = pkv.tile([128, SA], f32, tag="pkv")
            for (c0, c1) in ((0, 512), (512, SA)):
                dr_chain(psum_k[:, c0:c1], wkh3, wkl3, 0, 128, c0, c1)
            rope(psum_k, cq_sb[:, 0:SA], sq_sb[:, 0:SA], kT, SA, ropep)

            psum_v = pkv.tile([128, SA], f32, tag="pkv")
            for (c0, c1) in ((0, 512), (512, SA)):
                dr_chain(psum_v[:, c0:c1], wvh3, wvl3, 0, 128, c0, c1)
            vTe = ropep.tile([128, SA], bf16, tag="vTe")
            nc.scalar.copy(vTe, psum_v)
            for kc in range(SAC):
                nc.sync.dma_start(
                    out=vn[:, kc * 128:(kc + 1) * 128],
                    in_=vTe[:, kc * 128:(kc + 1) * 128],
                    transpose=True,
                )

        # -------- Q chains + attention heads, software-pipelined --------
        with tc.tile_pool(name="pq", bufs=2, space="PSUM") as pq, \
             tc.tile_pool(name="ropeq", bufs=2) as ropeq, \
             tc.tile_pool(name="ps", bufs=2, space="PSUM") as ps_p, \
             tc.tile_pool(name="po", bufs=1, space="PSUM") as po_p, \
             tc.tile_pool(name="pc", bufs=1, space="PSUM") as pc_p, \
             tc.tile_pool(name="ppool", bufs=2) as ppool, \
             tc.tile_pool(name="rpool", bufs=2) as rpool:

            def q_chain_emit(h):
                """Emit one Q-projection matmul per next(); rope DVE ops are
                emitted right after each half's chain completes so the rope
                overlaps the rest of the interleaved stream."""
                for qs in (0, 512):
                    psq = pq.tile([128, 512], f32, tag="pq", name=f"pq{h}_{qs}")
                    n = 0
                    last = 3 * (DC // 2) - 1
                    for (w3, x3) in ((wqh3, xh3), (wqh3, xl3), (wql3, xh3)):
                        for pr in range(DC // 2):
                            nc.tensor.matmul(
                                psq,
                                lhsT=w3[:, 2 * pr:2 * pr + 2, h * 128:(h + 1) * 128],
                                rhs=x3[:, 2 * pr:2 * pr + 2, qs:qs + 512],
                                start=(n == 0), stop=(n == last),
                                perf_mode=DR,
                            )
                            n += 1
                            yield None
                    rope(
                        psq, cq_sb[:, qs:qs + 512], sq_sb[:, qs:qs + 512],
                        qT[h][:, qs:qs + 512], 512, ropeq,
                    )

            def spans(kc):
                return [(0, 512), (512, 1024)] if qa_kc[kc] == 0 else [(512, 1024)]

            def b_head(h, psum_o, psum_c, filler):
                def fill(n):
                    for _ in range(n):
                        next(filler, None)

                def scores_exp(kc):
                    p_sb = ppool.tile([128, S], bf16, tag="p_sb", name=f"p{h}_{kc}")
                    for (s0, s1) in spans(kc):
                        psum_s = ps_p.tile([128, 512], f32, tag="ps", name=f"ps{h}_{kc}_{s0}")
                        nc.tensor.matmul(
                            psum_s[:, 0:s1 - s0],
                            lhsT=kT[:, kc * 128:(kc + 1) * 128],
                            rhs=qT[h][:, s0:s1],
                            start=True, stop=True,
                        )
                        fill(3)
                        nc.scalar.activation(
                            p_sb[:, s0:s1], psum_s[:, 0:s1 - s0], Exp, scale=SCALE / 1024.0
                        )
                    for (qt0, n, idx0) in runs_by_kc.get(kc, ()):
                        nc.vector.tensor_mul(
                            p_sb[:, qt0 * 128:(qt0 + n) * 128],
                            p_sb[:, qt0 * 128:(qt0 + n) * 128],
                            mk_sb[:, idx0 * 128:(idx0 + n) * 128],
                        )
                    return p_sb

                def reduce_chunk(kc, p_sb):
                    for (s0, s1) in spans(kc):
                        if s0 == 0:
                            start = (kc == span01_kcs[0])
                            stop = (kc == span01_kcs[-1])
                        else:
                            start = (kc == 0)
                            stop = (kc == SAC - 1)
                        nc.tensor.matmul(
                            psum_c[:, s0:s1], lhsT=ones128,
                            rhs=p_sb[:, s0:s1], start=start, stop=stop,
                        )
                        fill(2)
                        nc.tensor.matmul(
                            psum_o[:, s0:s1],
                            lhsT=vn[:, kc * 128:(kc + 1) * 128],
                            rhs=p_sb[:, s0:s1], start=start, stop=stop,
                        )
                        fill(2)

                prev = None
                for kc in range(SAC):
                    p_sb = scores_exp(kc)
                    fill(3)
                    if prev is not None:
                        reduce_chunk(prev[0], prev[1])
                    prev = (kc, p_sb)
                reduce_chunk(prev[0], prev[1])
                fill(64)  # drain any remaining interleaved Q matmuls

            def b_norm(h, psum_o, psum_c):
                rb = rpool.tile([128, S], f32, tag="rb", name=f"rb{h}")
                nc.vector.reciprocal_approx_fast(rb, psum_c)
                nc.vector.tensor_mul(attn[h], psum_o, rb)

            # Q0 runs un-interleaved (no attention head active yet)
            for _ in q_chain_emit(0):
                pass
            for h in range(NH):
                psum_o = po_p.tile([128, S], f32, tag="po", name=f"po{h}")
                psum_c = pc_p.tile([128, S], f32, tag="pc", name=f"pc{h}")
                filler = q_chain_emit(h + 1) if h + 1 < NH else iter(())
                b_head(h, psum_o, psum_c, filler)
                b_norm(h, psum_o, psum_c)

        # ================= Phase C: out^T = wo^T @ attn =================
        with tc.tile_pool(name="poc", bufs=2, space="PSUM") as poc, \
             tc.tile_pool(name="outp", bufs=3) as outp:
            for dc in range(DC):
                oc = poc.tile([128, S], f32, tag="oc", name=f"oc{dc}")
                for h in range(NH):
                    for qs in (0, 512):
                        nc.tensor.matmul(
                            oc[:, qs:qs + 512],
                            lhsT=wo_sb[:, h * D + dc * 128: h * D + (dc + 1) * 128],
                            rhs=attn[h][:, qs:qs + 512],
                            start=(h == 0), stop=(h == NH - 1),
                        )
                osb = outp.tile([128, S], bf16, tag="osb", name=f"osb{dc}")
                if dc % 2 == 0:
                    nc.scalar.copy(osb, oc)
                else:
                    nc.vector.tensor_copy(osb, oc)
                (nc.sync if dc % 2 == 0 else nc.gpsimd).dma_start(
                    out=out_d[:, dc * S:(dc + 1) * S], in_=osb
                )

    nc.compile()
    return nc


def _get_nc(meta):
    if meta not in _NC_CACHE:
        _NC_CACHE[meta] = _build_nc(meta)
    return _NC_CACHE[meta]


def _host_prep(hidden_states, cos, sin, wq, wk, wv, wo, position_ids, active_mask):
    import ml_dtypes

    bf16 = ml_dtypes.bfloat16
    f8 = ml_dtypes.float8_e4m3
    hs = np.asarray(hidden_states, dtype=np.float32)
    cos = np.asarray(cos, dtype=np.float32)
    sin = np.asarray(sin, dtype=np.float32)
    wq = np.asarray(wq, dtype=np.float32)
    wk = np.asarray(wk, dtype=np.float32)
    wv = np.asarray(wv, dtype=np.float32)
    wo = np.asarray(wo, dtype=np.float32)
    pos = np.asarray(position_ids).astype(np.int64)
    am = np.asarray(active_mask).astype(bool)
    B = hs.shape[0]
    assert B == 2 and hs.shape[1] == S and hs.shape[2] == D

    ar = np.arange(S)
    perms, pos_sels, nacts = [], [], []
    for b in range(B):
        # actives-first stable order == full q permutation; its prefix is the
        # compacted-key order
        perm = np.argsort(np.where(am[b], ar, ar + S), kind="stable")
        nact = int(am[b].sum())
        perms.append(perm)
        pos_sels.append(pos[b][perm[:nact]])
        nacts.append(nact)

    SAC = int(max((n + 127) // 128 for n in nacts))
    SA = SAC * 128

    # tile structure in (sorted-key, permuted-q) space, unioned over batches
    live = np.zeros((SAC, 8), dtype=bool)
    full = np.ones((SAC, 8), dtype=bool)
    for b in range(B):
        ps = pos_sels[b]
        n = nacts[b]
        qpos = pos[b][perms[b]]
        qmax = qpos.reshape(8, 128).max(axis=1)
        qmin = qpos.reshape(8, 128).min(axis=1)
        for kc in range(SAC):
            ks, ke = kc * 128, min(kc * 128 + 128, n)
            for qt in range(8):
                if ks >= n:
                    full[kc, qt] = False
                    continue
                l = ps[ks] <= qmax[qt]
                f = (ke - ks == 128) and (ps[ke - 1] <= qmin[qt])
                live[kc, qt] |= l
                if not (l and f):
                    full[kc, qt] = False

    qt_min = [int(np.argmax(live[kc])) if live[kc].any() else 8 for kc in range(SAC)]
    qa_kc = tuple(0 if q < 4 else 512 for q in qt_min)
    span01_kcs = tuple(kc for kc in range(SAC) if qa_kc[kc] == 0)

    mask_list = []
    for kc in range(SAC):
        for qt in range(qa_kc[kc] // 128, 8):
            if not full[kc, qt]:
                mask_list.append((kc, qt))
    mask_runs = []
    idx = 0
    i = 0
    while i < len(mask_list):
        kc, qt0 = mask_list[i]
        n = 1
        while (i + n < len(mask_list) and mask_list[i + n] == (kc, qt0 + n)):
            n += 1
        mask_runs.append((kc, qt0, n, idx))
        idx += n
        i += n
    mask_runs = tuple(mask_runs)
    meta = (SAC, qa_kc, span01_kcs, mask_runs, idx)
    NSTR = max(1, idx)

    s2 = np.concatenate([sin.T[64:], -sin.T[:64]], axis=0)  # [HD, S] table

    def chunked(a, nchunks):
        F = a.shape[1]
        return np.ascontiguousarray(
            a.reshape(nchunks, 128, F).transpose(1, 0, 2).reshape(128, nchunks * F)
        )

    in_maps = []
    for core in range(8):
        b, g = divmod(core, 4)
        n = nacts[b]
        ps = pos_sels[b]
        xperm = hs[b][perms[b]]         # [S, D] rows in permuted-q order
        qpos = pos[b][perms[b]]

        cqb = cos.T[:, qpos]            # rope tables gathered to permuted q
        sqb = s2[:, qpos]

        mk = np.zeros((128, NSTR * 128), dtype=np.float32)
        kidx = np.arange(128)
        for (kc, qt0, nt, idx0) in mask_runs:
            for j in range(nt):
                qt = qt0 + j
                ks = kc * 128
                kvalid = (ks + kidx) < n
                kp = ps[np.minimum(ks + kidx, max(n - 1, 0))]
                qp = qpos[qt * 128:(qt + 1) * 128]
                mk[:, (idx0 + j) * 128:(idx0 + j + 1) * 128] = (
                    kvalid[:, None] & (kp[:, None] <= qp[None, :])
                ).astype(np.float32)

        def hilo(a, nch):
            hi = a.astype(f8)
            lo = (a - hi.astype(np.float32)).astype(f8)
            return chunked(hi, nch), chunked(lo, nch)

        # q/k/v weights are host-scaled by 32 to sit in fp8e4's normal range;
        # the q*k factor (32*32) is folded into the exp scale on device and
        # the v factor into wo here.
        xhc, xlc = hilo(xperm.T, DC)
        wqhc, wqlc = hilo(32.0 * wq[:, g * 512:(g + 1) * 512], DC)
        wkhc, wklc = hilo(32.0 * wk[:, g * 128:(g + 1) * 128], DC)
        wvhc, wvlc = hilo(32.0 * wv[:, g * 128:(g + 1) * 128], DC)
        in_maps.append({
            "xh": xhc, "xl": xlc,
            "wqh": wqhc, "wql": wqlc,
            "wkh": wkhc, "wkl": wklc,
            "wvh": wvhc, "wvl": wvlc,
            "wo": chunked((wo[g * 512:(g + 1) * 512] / 32.0).astype(bf16), NH),
            "cq": cqb.astype(bf16), "sq": sqb.astype(bf16),
            "mk": mk.astype(bf16),
        })
    return meta, perms, in_maps


def kernel(hidden_states, cos, sin, wq, wk, wv, wo, position_ids, active_mask):
    global LAST_EXEC_NS, LAST_RESULTS
    from concourse.bass_utils import run_bass_kernel_spmd

    meta, perms, in_maps = _host_prep(
        hidden_states, cos, sin, wq, wk, wv, wo, position_ids, active_mask
    )
    nc = _get_nc(meta)
    res = run_bass_kernel_spmd(nc, in_maps, core_ids=list(range(8)), trace=TRACE)
    LAST_EXEC_NS = res.exec_time_ns
    LAST_RESULTS = res
    B = np.asarray(hidden_states).shape[0]
    full = np.zeros((B, S, D), dtype=np.float32)
    for core in range(8):
        b = core // 4
        o = np.asarray(res.results[core]["out"]).astype(np.float32)
        outT = o.reshape(128, DC, S).transpose(1, 0, 2).reshape(D, S)
        full[b][perms[b]] += outT.T
    return full


# revision 18
# speedup vs baseline: 1.1545x; 1.1545x over previous
"""Trainium2 Bass kernel for MoRAttention (sparse selective-KV GQA attention).

Math: the reference's argsort/gather of active keys == dense attention with
mask = active[k] & (pos[k] <= pos[q]) (softmax is permutation invariant along
keys; q columns are independent). We permute the q axis on the host to
actives-first order, so the first SA columns of the permuted hidden state ARE
the compacted keys: K/V projections read a prefix of the same SBUF tensor the
Q projection reads — no separate gathered-x input. The host inverse-permutes
the output columns.

Causal structure over sorted keys/queries lets us statically skip dead
(k-chunk, q-span) tiles; tiles that are computed but not fully valid get a
multiplicative mask precomputed on the host.

Sharding: 8 cores = 2 batches x 4 kv-groups. Core (b, g) computes q-heads
[4g, 4g+4) + kv-head g of batch b, producing a partial o_proj output
out^T [D, S]; the host sums the 4 partials per batch.

All matmul operands bf16 (same PE rate as f32r, half the DMA/SBUF traffic),
f32 PSUM. Pipeline: K, V, Q0, B0|Q1, B1|Q2, B2|Q3, B3, C with attnV/colsum
staggered one k-chunk behind scores so PE never waits on exp, and input DMAs
striped across the three DGE rings (SP, Act, Pool) in consumption order.
"""

import numpy as np

S, D, HD = 1024, 2048, 128
NH = 4           # q heads per core
DC = D // 128    # contraction chunks
SCALE = HD ** -0.5

TRACE = False
LAST_EXEC_NS = None
LAST_RESULTS = None

_NC_CACHE = {}


def _build_nc(meta):
    import concourse.mybir as mybir
    from concourse import bacc
    from concourse.tile import TileContext
    from contextlib import ExitStack

    SAC, qa_kc, span01_kcs, mask_runs, nstr = meta
    SA = SAC * 128
    NSTR = max(1, nstr)
    runs_by_kc = {}
    for (kc, qt0, n, idx0) in mask_runs:
        runs_by_kc.setdefault(kc, []).append((qt0, n, idx0))

    f32 = mybir.dt.float32
    bf16 = mybir.dt.bfloat16
    Exp = mybir.ActivationFunctionType.Exp

    nc = bacc.Bacc("TRN2", target_bir_lowering=False, debug=False)

    xs_d = nc.dram_tensor("xs", [128, DC * S], bf16, kind="ExternalInput")
    wq_d = nc.dram_tensor("wq", [128, DC * 512], bf16, kind="ExternalInput")
    wk_d = nc.dram_tensor("wk", [128, DC * 128], bf16, kind="ExternalInput")
    wv_d = nc.dram_tensor("wv", [128, DC * 128], bf16, kind="ExternalInput")
    wo_d = nc.dram_tensor("wo", [128, NH * D], bf16, kind="ExternalInput")
    cq_d = nc.dram_tensor("cq", [128, S], bf16, kind="ExternalInput")
    sq_d = nc.dram_tensor("sq", [128, S], bf16, kind="ExternalInput")
    mk_d = nc.dram_tensor("mk", [128, NSTR * 128], bf16, kind="ExternalInput")
    out_d = nc.dram_tensor("out", [128, DC * S], bf16, kind="ExternalOutput")

    with TileContext(nc) as tc, ExitStack() as ctx:
        singles = ctx.enter_context(tc.tile_pool(name="singles", bufs=1))
        persist = ctx.enter_context(tc.tile_pool(name="persist", bufs=1))

        ones_tmp = singles.tile([128, 128], f32)
        nc.vector.memset(ones_tmp, 1.0)
        ones128 = singles.tile([128, 128], bf16)
        nc.vector.tensor_copy(ones128, ones_tmp)

        xs_sb = persist.tile([128, DC * S], bf16, tag="xs")
        wq_sb = persist.tile([128, DC * 512], bf16, tag="wq")
        wk_sb = persist.tile([128, DC * 128], bf16, tag="wk")
        wv_sb = persist.tile([128, DC * 128], bf16, tag="wv")
        wo_sb = persist.tile([128, NH * D], bf16, tag="wo")
        cq_sb = persist.tile([128, S], bf16, tag="cq")
        sq_sb = persist.tile([128, S], bf16, tag="sq")
        mk_sb = persist.tile([128, NSTR * 128], bf16, tag="mk")

        kT = persist.tile([128, SA], bf16, tag="kT")
        vn = persist.tile([128, SA], bf16, tag="vn")
        qT = [persist.tile([128, S], bf16, tag=f"qT{h}", name=f"qT{h}") for h in range(NH)]
        attn = [persist.tile([128, S], bf16, tag=f"attn{h}", name=f"attn{h}") for h in range(NH)]

        # ---- input DMAs striped across the 3 DGE rings by consumption order.
        # K/V chains consume xs[:, dc*S : dc*S+SA] (the compacted-key prefix of
        # each chunk) first; Q chains then read the full chunks; wq is needed
        # from the Q0 chain on.
        # hwdge rings (sync/scalar) carry everything consumed early; the
        # software-DGE gpsimd ring only carries late-consumed bulk (q-only
        # chunk tails, o-proj weights).
        hw = [nc.sync, nc.scalar]
        HW4 = DC * 128 // 4
        for i in range(4):  # wk quarters, alternating rings
            hw[i % 2].dma_start(
                out=wk_sb[:, i * HW4:(i + 1) * HW4], in_=wk_d[:, i * HW4:(i + 1) * HW4]
            )
        for dc in range(0, 4):  # first key-prefix parts right behind wk
            hw[dc % 2].dma_start(
                out=xs_sb[:, dc * S: dc * S + SA], in_=xs_d[:, dc * S: dc * S + SA]
            )
        for i in range(4):
            hw[i % 2].dma_start(
                out=wv_sb[:, i * HW4:(i + 1) * HW4], in_=wv_d[:, i * HW4:(i + 1) * HW4]
            )
        for dc in range(4, DC):  # remaining key-prefix parts
            hw[dc % 2].dma_start(
                out=xs_sb[:, dc * S: dc * S + SA], in_=xs_d[:, dc * S: dc * S + SA]
            )
        nc.sync.dma_start(out=cq_sb, in_=cq_d[:, :])
        nc.scalar.dma_start(out=sq_sb, in_=sq_d[:, :])
        nc.scalar.dma_start(out=mk_sb, in_=mk_d[:, :])
        for dc in range(DC):  # q-only tails of each chunk
            nc.gpsimd.dma_start(
                out=xs_sb[:, dc * S + SA: (dc + 1) * S],
                in_=xs_d[:, dc * S + SA: (dc + 1) * S],
            )
        for i in range(4):  # q-proj weights
            w = DC * 512 // 4
            hw[i % 2].dma_start(
                out=wq_sb[:, i * w:(i + 1) * w], in_=wq_d[:, i * w:(i + 1) * w]
            )
        for i in range(2):  # o-proj weights, needed only in phase C
            w = NH * D // 2
            nc.gpsimd.dma_start(
                out=wo_sb[:, i * w:(i + 1) * w], in_=wo_d[:, i * w:(i + 1) * w]
            )

        def rope(psum, cos_t, sin_t, dst, w, pool):
            # dst = psum*cos + rot_half(psum)*sin2  (sin2 pre-arranged so a
            # plain half-swap after the multiply gives rot_half()*sin)
            pc = pool.tile([128, w], bf16, tag="ropec")
            ps_ = pool.tile([128, w], bf16, tag="ropes")
            pw = pool.tile([128, w], bf16, tag="ropew")
            nc.vector.tensor_mul(pc, psum, cos_t)
            nc.vector.tensor_mul(ps_, psum, sin_t)
            nc.sync.dma_start(out=pw[0:64, :], in_=ps_[64:128, :])
            nc.sync.dma_start(out=pw[64:128, :], in_=ps_[0:64, :])
            nc.vector.tensor_add(dst, pc, pw)

        # ================= Phase A: K, V =================
        with tc.tile_pool(name="pkv", bufs=2, space="PSUM") as pkv, \
             tc.tile_pool(name="ropep", bufs=2) as ropep:
            psum_k = pkv.tile([128, SA], f32, tag="pkv")
            for (c0, c1) in ((0, 512), (512, SA)):
                for dc in range(DC):
                    nc.tensor.matmul(
                        psum_k[:, c0:c1],
                        lhsT=wk_sb[:, dc * 128:(dc + 1) * 128],
                        rhs=xs_sb[:, dc * S + c0: dc * S + c1],
                        start=(dc == 0), stop=(dc == DC - 1),
                    )
            rope(psum_k, cq_sb[:, 0:SA], sq_sb[:, 0:SA], kT, SA, ropep)

            psum_v = pkv.tile([128, SA], f32, tag="pkv")
            for (c0, c1) in ((0, 512), (512, SA)):
                for dc in range(DC):
                    nc.tensor.matmul(
                        psum_v[:, c0:c1],
                        lhsT=wv_sb[:, dc * 128:(dc + 1) * 128],
                        rhs=xs_sb[:, dc * S + c0: dc * S + c1],
                        start=(dc == 0), stop=(dc == DC - 1),
                    )
            vTe = ropep.tile([128, SA], bf16, tag="vTe")
            nc.scalar.copy(vTe, psum_v)
            for kc in range(SAC):
                nc.sync.dma_start(
                    out=vn[:, kc * 128:(kc + 1) * 128],
                    in_=vTe[:, kc * 128:(kc + 1) * 128],
                    transpose=True,
                )

        # -------- Q chains + attention heads, software-pipelined --------
        with tc.tile_pool(name="pq", bufs=2, space="PSUM") as pq, \
             tc.tile_pool(name="ropeq", bufs=2) as ropeq, \
             tc.tile_pool(name="ps", bufs=2, space="PSUM") as ps_p, \
             tc.tile_pool(name="po", bufs=1, space="PSUM") as po_p, \
             tc.tile_pool(name="pc", bufs=1, space="PSUM") as pc_p, \
             tc.tile_pool(name="ppool", bufs=2) as ppool, \
             tc.tile_pool(name="rpool", bufs=2) as rpool:

            def q_chain_emit(h):
                """Emit one Q-projection matmul per next(); rope DVE ops are
                emitted right after each half's chain completes so the rope
                overlaps the rest of the interleaved stream."""
                for qs in (0, 512):
                    psq = pq.tile([128, 512], f32, tag="pq", name=f"pq{h}_{qs}")
                    for dc in range(DC):
                        nc.tensor.matmul(
                            psq,
                            lhsT=wq_sb[:, dc * 512 + h * 128: dc * 512 + (h + 1) * 128],
                            rhs=xs_sb[:, dc * S + qs: dc * S + qs + 512],
                            start=(dc == 0), stop=(dc == DC - 1),
                        )
                        yield None
                    rope(
                        psq, cq_sb[:, qs:qs + 512], sq_sb[:, qs:qs + 512],
                        qT[h][:, qs:qs + 512], 512, ropeq,
                    )

            def spans(kc):
                return [(0, 512), (512, 1024)] if qa_kc[kc] == 0 else [(512, 1024)]

            def b_head(h, psum_o, psum_c, filler):
                def fill(n):
                    for _ in range(n):
                        next(filler, None)

                def scores_exp(kc):
                    p_sb = ppool.tile([128, S], bf16, tag="p_sb", name=f"p{h}_{kc}")
                    for (s0, s1) in spans(kc):
                        psum_s = ps_p.tile([128, 512], f32, tag="ps", name=f"ps{h}_{kc}_{s0}")
                        nc.tensor.matmul(
                            psum_s[:, 0:s1 - s0],
                            lhsT=kT[:, kc * 128:(kc + 1) * 128],
                            rhs=qT[h][:, s0:s1],
                            start=True, stop=True,
                        )
                        fill(3)
                        nc.scalar.activation(
                            p_sb[:, s0:s1], psum_s[:, 0:s1 - s0], Exp, scale=SCALE
                        )
                    for (qt0, n, idx0) in runs_by_kc.get(kc, ()):
                        nc.vector.tensor_mul(
                            p_sb[:, qt0 * 128:(qt0 + n) * 128],
                            p_sb[:, qt0 * 128:(qt0 + n) * 128],
                            mk_sb[:, idx0 * 128:(idx0 + n) * 128],
                        )
                    return p_sb

                def reduce_chunk(kc, p_sb):
                    for (s0, s1) in spans(kc):
                        if s0 == 0:
                            start = (kc == span01_kcs[0])
                            stop = (kc == span01_kcs[-1])
                        else:
                            start = (kc == 0)
                            stop = (kc == SAC - 1)
                        nc.tensor.matmul(
                            psum_c[:, s0:s1], lhsT=ones128,
                            rhs=p_sb[:, s0:s1], start=start, stop=stop,
                        )
                        fill(2)
                        nc.tensor.matmul(
                            psum_o[:, s0:s1],
                            lhsT=vn[:, kc * 128:(kc + 1) * 128],
                            rhs=p_sb[:, s0:s1], start=start, stop=stop,
                        )
                        fill(2)

                prev = None
                for kc in range(SAC):
                    p_sb = scores_exp(kc)
                    fill(6 if kc == 0 else 3)
                    if prev is not None:
                        reduce_chunk(prev[0], prev[1])
                    prev = (kc, p_sb)
                reduce_chunk(prev[0], prev[1])
                fill(64)  # drain any remaining interleaved Q matmuls

            def b_norm(h, psum_o, psum_c):
                rb = rpool.tile([128, S], f32, tag="rb", name=f"rb{h}")
                nc.vector.reciprocal_approx_fast(rb, psum_c)
                nc.vector.tensor_mul(attn[h], psum_o, rb)

            # Q0 runs un-interleaved (no attention head active yet)
            for _ in q_chain_emit(0):
                pass
            for h in range(NH):
                psum_o = po_p.tile([128, S], f32, tag="po", name=f"po{h}")
                psum_c = pc_p.tile([128, S], f32, tag="pc", name=f"pc{h}")
                filler = q_chain_emit(h + 1) if h + 1 < NH else iter(())
                b_head(h, psum_o, psum_c, filler)
                b_norm(h, psum_o, psum_c)

        # ================= Phase C: out^T = wo^T @ attn =================
        with tc.tile_pool(name="poc", bufs=2, space="PSUM") as poc, \
             tc.tile_pool(name="outp", bufs=3) as outp:
            for dc in range(DC):
                oc = poc.tile([128, S], f32, tag="oc", name=f"oc{dc}")
                for h in range(NH):
                    for qs in (0, 512):
                        nc.tensor.matmul(
                            oc[:, qs:qs + 512],
                            lhsT=wo_sb[:, h * D + dc * 128: h * D + (dc + 1) * 128],
                            rhs=attn[h][:, qs:qs + 512],
                            start=(h == 0), stop=(h == NH - 1),
                        )
                osb = outp.tile([128, S], bf16, tag="osb", name=f"osb{dc}")
                if dc % 2 == 0:
                    nc.scalar.copy(osb, oc)
                else:
                    nc.vector.tensor_copy(osb, oc)
                (nc.sync if dc % 2 == 0 else nc.gpsimd).dma_start(
                    out=out_d[:, dc * S:(dc + 1) * S], in_=osb
                )

    nc.compile()
    return nc


def _get_nc(meta):
    if meta not in _NC_CACHE:
        _NC_CACHE[meta] = _build_nc(meta)
    return _NC_CACHE[meta]


def _host_prep(hidden_states, cos, sin, wq, wk, wv, wo, position_ids, active_mask):
    import ml_dtypes

    bf16 = ml_dtypes.bfloat16
    hs = np.asarray(hidden_states, dtype=np.float32)
    cos = np.asarray(cos, dtype=np.float32)
    sin = np.asarray(sin, dtype=np.float32)
    wq = np.asarray(wq, dtype=np.float32)
    wk = np.asarray(wk, dtype=np.float32)
    wv = np.asarray(wv, dtype=np.float32)
    wo = np.asarray(wo, dtype=np.float32)
    pos = np.asarray(position_ids).astype(np.int64)
    am = np.asarray(active_mask).astype(bool)
    B = hs.shape[0]
    assert B == 2 and hs.shape[1] == S and hs.shape[2] == D

    ar = np.arange(S)
    perms, pos_sels, nacts = [], [], []
    for b in range(B):
        # actives-first stable order == full q permutation; its prefix is the
        # compacted-key order
        perm = np.argsort(np.where(am[b], ar, ar + S), kind="stable")
        nact = int(am[b].sum())
        perms.append(perm)
        pos_sels.append(pos[b][perm[:nact]])
        nacts.append(nact)

    SAC = int(max((n + 127) // 128 for n in nacts))
    SA = SAC * 128

    # tile structure in (sorted-key, permuted-q) space, unioned over batches
    live = np.zeros((SAC, 8), dtype=bool)
    full = np.ones((SAC, 8), dtype=bool)
    for b in range(B):
        ps = pos_sels[b]
        n = nacts[b]
        qpos = pos[b][perms[b]]
        qmax = qpos.reshape(8, 128).max(axis=1)
        qmin = qpos.reshape(8, 128).min(axis=1)
        for kc in range(SAC):
            ks, ke = kc * 128, min(kc * 128 + 128, n)
            for qt in range(8):
                if ks >= n:
                    full[kc, qt] = False
                    continue
                l = ps[ks] <= qmax[qt]
                f = (ke - ks == 128) and (ps[ke - 1] <= qmin[qt])
                live[kc, qt] |= l
                if not (l and f):
                    full[kc, qt] = False

    qt_min = [int(np.argmax(live[kc])) if live[kc].any() else 8 for kc in range(SAC)]
    qa_kc = tuple(0 if q < 4 else 512 for q in qt_min)
    span01_kcs = tuple(kc for kc in range(SAC) if qa_kc[kc] == 0)

    mask_list = []
    for kc in range(SAC):
        for qt in range(qa_kc[kc] // 128, 8):
            if not full[kc, qt]:
                mask_list.append((kc, qt))
    mask_runs = []
    idx = 0
    i = 0
    while i < len(mask_list):
        kc, qt0 = mask_list[i]
        n = 1
        while (i + n < len(mask_list) and mask_list[i + n] == (kc, qt0 + n)):
            n += 1
        mask_runs.append((kc, qt0, n, idx))
        idx += n
        i += n
    mask_runs = tuple(mask_runs)
    meta = (SAC, qa_kc, span01_kcs, mask_runs, idx)
    NSTR = max(1, idx)

    s2 = np.concatenate([sin.T[64:], -sin.T[:64]], axis=0)  # [HD, S] table

    def chunked(a, nchunks):
        F = a.shape[1]
        return np.ascontiguousarray(
            a.reshape(nchunks, 128, F).transpose(1, 0, 2).reshape(128, nchunks * F)
        )

    in_maps = []
    for core in range(8):
        b, g = divmod(core, 4)
        n = nacts[b]
        ps = pos_sels[b]
        xperm = hs[b][perms[b]]         # [S, D] rows in permuted-q order
        qpos = pos[b][perms[b]]

        cqb = cos.T[:, qpos]            # rope tables gathered to permuted q
        sqb = s2[:, qpos]

        mk = np.zeros((128, NSTR * 128), dtype=np.float32)
        kidx = np.arange(128)
        for (kc, qt0, nt, idx0) in mask_runs:
            for j in range(nt):
                qt = qt0 + j
                ks = kc * 128
                kvalid = (ks + kidx) < n
                kp = ps[np.minimum(ks + kidx, max(n - 1, 0))]
                qp = qpos[qt * 128:(qt + 1) * 128]
                mk[:, (idx0 + j) * 128:(idx0 + j + 1) * 128] = (
                    kvalid[:, None] & (kp[:, None] <= qp[None, :])
                ).astype(np.float32)

        in_maps.append({
            "xs": chunked(xperm.T.astype(bf16), DC),
            "wq": chunked(wq[:, g * 512:(g + 1) * 512].astype(bf16), DC),
            "wk": chunked(wk[:, g * 128:(g + 1) * 128].astype(bf16), DC),
            "wv": chunked(wv[:, g * 128:(g + 1) * 128].astype(bf16), DC),
            "wo": chunked(wo[g * 512:(g + 1) * 512].astype(bf16), NH),
            "cq": cqb.astype(bf16), "sq": sqb.astype(bf16),
            "mk": mk.astype(bf16),
        })
    return meta, perms, in_maps


def kernel(hidden_states, cos, sin, wq, wk, wv, wo, position_ids, active_mask):
    global LAST_EXEC_NS, LAST_RESULTS
    from concourse.bass_utils import run_bass_kernel_spmd

    meta, perms, in_maps = _host_prep(
        hidden_states, cos, sin, wq, wk, wv, wo, position_ids, active_mask
    )
    nc = _get_nc(meta)
    res = run_bass_kernel_spmd(nc, in_maps, core_ids=list(range(8)), trace=TRACE)
    LAST_EXEC_NS = res.exec_time_ns
    LAST_RESULTS = res
    B = np.asarray(hidden_states).shape[0]
    full = np.zeros((B, S, D), dtype=np.float32)
    for core in range(8):
        b = core // 4
        o = np.asarray(res.results[core]["out"]).astype(np.float32)
        outT = o.reshape(128, DC, S).transpose(1, 0, 2).reshape(D, S)
        full[b][perms[b]] += outT.T
    return full
